# Initial kernel scaffold
#
"""Trainium2 Bass kernel for nn_EncoderLayer_64175401337444 (sparse_attention).

Strategy (8 NeuronCores, data-parallel over patches):
  The whole layer is pointwise-per-token EXCEPT the per-patch attention, and
  the serialized `order` gather / `inverse` scatter are inverse permutations.
  So we run the entire front half (LN1 -> QKV -> patch attention -> proj) in
  SERIALIZED order: core c owns serialized positions [c*16384, (c+1)*16384)
  = 128 patches, gathering its rows from a replicated feat buffer with
  indirect DMA.  The attention branch output ("delta", pre-residual) is then
  exchanged to ORIGINAL row order with a single on-device AllToAll (host
  precomputes the routing from `order`), after which the back half
  (residual -> LN2 -> MLP -> residual) runs on each core's contiguous slab of
  original rows, and each core emits its contiguous slice of the final output.

  LN gains/biases are folded into the adjacent matmul weights on the host;
  matmuls run in bf16 with f32 PSUM accumulation; LN statistics, softmax
  normalization, residuals and the exchanged delta stay f32.
"""
import sys

sys.path.insert(0, "/opt/trn_rl_repo")

import numpy as np

import concourse.bass as bass
import concourse.tile as tile
from concourse import mybir
from concourse.bass_utils import run_bass_kernel_spmd
from concourse.masks import make_identity

# ---------------------------------------------------------------------------
# Workaround for this walrus build accepting at most ONE sync wait per
# instruction: after Tile finishes scheduling, split any multi-wait
# instruction into single-wait same-engine NoOps placed immediately before it.
_uid = [0]


def _split_multi_waits(nc):
    register = getattr(nc, "register_instruction", None)
    for fn in nc.m.functions:
        for bb in fn.blocks:
            insts = bb.instructions
            if not any(
                i.sync_info is not None and len(i.sync_info.on_wait) > 1
                for i in insts
            ):
                continue
            new_list = []
            for inst in insts:
                si = inst.sync_info
                if si is not None and len(si.on_wait) > 1:
                    waits = list(si.on_wait)
                    for w in waits[:-1]:
                        _uid[0] += 1
                        nop = mybir.InstNoOp(
                            name=f"I-waitsplit-{_uid[0]}", ins=[], outs=[]
                        )
                        nop.engine = inst.engine
                        nop.sync_info = mybir.SyncInfo(on_wait=[w], on_update=[])
                        if register is not None:
                            register(nop, overwrite=True)
                        new_list.append(nop)
                    inst.sync_info = mybir.SyncInfo(
                        on_wait=[waits[-1]], on_update=list(si.on_update)
                    )
                new_list.append(inst)
            bb.instructions.clear()
            for inst in new_list:
                bb.instructions.append(inst)


if not getattr(tile.TileContext, "_wait_split_patched", False):
    _orig_dab = tile.TileContext._drain_and_barrier

    def _dab_patched(self, tick_clock, wait_clock):
        _orig_dab(self, tick_clock, wait_clock)
        _split_multi_waits(self.nc)

    tile.TileContext._drain_and_barrier = _dab_patched
    tile.TileContext._wait_split_patched = True

# ---------------------------------------------------------------------------

N = 131072
C = 256
H = 8
K = 128          # patch size == SBUF partition count
HID = 1024
NCORE = 8
S = N // NCORE   # 16384 rows per core
PPC = S // K     # 128 patches per core
DH = C // H      # 32
SCALE = DH ** -0.5
LN_EPS = 1e-5
GELU_FUNC = mybir.ActivationFunctionType.Gelu

F32 = mybir.dt.float32
BF16 = mybir.dt.bfloat16
I32 = mybir.dt.int32

_prog_cache = {}


def _build_program(bucket, ppc_a=PPC, ppc_b=PPC, use_a2a=True, do_a=True, do_b=True, bz=True, sg=16):
    tot = NCORE * bucket
    nc = bass.Bass()

    feat = nc.declare_dram_parameter("feat", [N, C], F32, isOutput=False)
    feat_slab = nc.declare_dram_parameter("feat_slab", [S, C], F32, isOutput=False)
    ordi = nc.declare_dram_parameter("ordi", [S], I32, isOutput=False)
    spos = nc.declare_dram_parameter("spos", [S], I32, isOutput=False)
    gidx = nc.declare_dram_parameter("gidx", [S], I32, isOutput=False)
    wqkT = nc.declare_dram_parameter("wqkT", [C, 512], BF16, isOutput=False)
    wvT = nc.declare_dram_parameter("wvT", [C, C], BF16, isOutput=False)
    bqk = nc.declare_dram_parameter("bqk", [K, 4], F32, isOutput=False)
    bv = nc.declare_dram_parameter("bv", [C], F32, isOutput=False)
    wpT = nc.declare_dram_parameter("wpT", [C, C], BF16, isOutput=False)
    pb = nc.declare_dram_parameter("pb", [C], F32, isOutput=False)
    w1T = nc.declare_dram_parameter("w1T", [C, HID], BF16, isOutput=False)
    b1 = nc.declare_dram_parameter("b1", [K, 8], F32, isOutput=False)
    w2T = nc.declare_dram_parameter("w2T", [HID, C], BF16, isOutput=False)
    b2 = nc.declare_dram_parameter("b2", [C], F32, isOutput=False)
    e128 = nc.declare_dram_parameter("e128", [K, 2, K], BF16, isOutput=False)
    out = nc.declare_dram_parameter("out", [S, C], F32, isOutput=True)

    send = nc.dram_tensor("send", [tot, C], F32)
    recv = nc.dram_tensor("recv", [tot, C], F32)

    ordi2 = ordi.rearrange("(p k) -> p k", k=K)
    spos2 = spos.rearrange("(p k) -> p k", k=K)
    gidx2 = gidx.rearrange("(p k) -> p k", k=K)

    with tile.TileContext(nc) as tc:
        with tc.tile_pool(name="consts", bufs=1) as consts:
            wqkT_sb = consts.tile([K, 2, 512], BF16)
            nc.sync.dma_start(out=wqkT_sb[:], in_=wqkT.rearrange("(k p) f -> p k f", p=K))
            wvT_sb = consts.tile([K, 2, C], BF16)
            nc.sync.dma_start(out=wvT_sb[:], in_=wvT.rearrange("(k p) f -> p k f", p=K))
            wpT_sb = consts.tile([K, 2, C], BF16)
            nc.sync.dma_start(out=wpT_sb[:], in_=wpT.rearrange("(k p) f -> p k f", p=K))
            w1T_sb = consts.tile([K, 2, HID], BF16)
            nc.sync.dma_start(out=w1T_sb[:], in_=w1T.rearrange("(k p) f -> p k f", p=K))
            w2T_sb = consts.tile([K, 8, C], BF16)
            nc.sync.dma_start(out=w2T_sb[:], in_=w2T.rearrange("(k p) f -> p k f", p=K))
            bqk_sb = consts.tile([K, 4], F32)
            nc.sync.dma_start(out=bqk_sb[:], in_=bqk[:])
            b1_sb = consts.tile([K, 8], F32)
            nc.sync.dma_start(out=b1_sb[:], in_=b1[:])
            e128_sb = consts.tile([K, 2, K], BF16)
            nc.sync.dma_start(out=e128_sb[:], in_=e128[:])

            def _bcast(handle):
                a = handle[:]
                return bass.AP(tensor=a.tensor, offset=a.offset, ap=[[0, K]] + list(a.ap))

            bv_sb = consts.tile([K, C], F32)
            nc.sync.dma_start(out=bv_sb[:], in_=_bcast(bv))
            pb_sb = consts.tile([K, C], F32)
            nc.sync.dma_start(out=pb_sb[:], in_=_bcast(pb))
            b2_sb = consts.tile([K, C], F32)
            nc.sync.dma_start(out=b2_sb[:], in_=_bcast(b2))
            ident = consts.tile([K, K], BF16)
            make_identity(nc, ident[:])
            eps_t = consts.tile([K, 1], F32)
            nc.vector.memset(eps_t[:], LN_EPS)
            ones_col = consts.tile([K, 1], BF16)
            nc.vector.memset(ones_col[:], 1.0)

            # ================= Phase A: serialized patches =================
            with (
                tc.tile_pool(name="pa_sm", bufs=6) as pa_sm,
                tc.tile_pool(name="pa_feat", bufs=2 * sg) as pa_feat,
                tc.tile_pool(name="pa_xn", bufs=2 * sg) as pa_xn,
                tc.tile_pool(name="pa_qk", bufs=2) as pa_qk,
                tc.tile_pool(name="pa_v", bufs=2) as pa_v,
                tc.tile_pool(name="pa_p", bufs=2) as pa_p,
                tc.tile_pool(name="pa_r", bufs=2) as pa_r,
                tc.tile_pool(name="pa_ao", bufs=2) as pa_ao,
                tc.tile_pool(name="pa_d", bufs=2) as pa_d,
                tc.tile_pool(name="ps_xt", bufs=1, space="PSUM") as ps_xt,
                tc.tile_pool(name="ps_qk", bufs=1, space="PSUM") as ps_qk,
                tc.tile_pool(name="ps_v", bufs=1, space="PSUM") as ps_v,
                tc.tile_pool(name="ps_sT", bufs=1, space="PSUM") as ps_sT,
                tc.tile_pool(name="ps_r", bufs=1, space="PSUM") as ps_r,
                tc.tile_pool(name="ps_ao", bufs=1, space="PSUM") as ps_ao,
                tc.tile_pool(name="ps_d", bufs=1, space="PSUM") as ps_d,
            ):
                n_a = ppc_a if do_a else 0

                def _a1_gather(p, i, st):
                        idx = pa_sm.tile([K, 1], I32)
                        nc.sync.dma_start(out=idx[:, 0:1], in_=ordi2[p, :].rearrange("(p one) -> p one", one=1))
                        feat_s = pa_feat.tile([K, C], F32)
                        nc.gpsimd.indirect_dma_start(
                            out=feat_s[:], out_offset=None, in_=feat[:],
                            in_offset=bass.IndirectOffsetOnAxis(ap=idx[:, :1], axis=0),
                        )
                        stats = pa_sm.tile([K, 6], F32)
                        nc.vector.bn_stats(out=stats[:], in_=feat_s[:])
                        nc.vector.bn_aggr(out=st["mvs"][:, i, :], in_=stats[:])
                        st["feat"][p] = feat_s

                def _a1_norm(p, i, st, xn_tiles):
                        xn = pa_xn.tile([K, C], BF16, name="xn")
                        nc.vector.tensor_scalar(
                            out=xn[:], in0=st["feat"][p][:], scalar1=st["mvs"][:, i, 0:1], scalar2=st["rstds"][:, i:i + 1],
                            op0=mybir.AluOpType.subtract, op1=mybir.AluOpType.mult,
                        )
                        xn_tiles[p] = xn

                def _a2_body(p, xn_tiles):
                        xn = xn_tiles[p]
                        xt_ps = ps_xt.tile([K, C], BF16, space="PSUM")
                        nc.tensor.transpose(out=xt_ps[:, 0:K], in_=xn[:, 0:K], identity=ident[:])
                        nc.tensor.transpose(out=xt_ps[:, K:C], in_=xn[:, K:C], identity=ident[:])
                        xt_bf = pa_p.tile([K, 2, K], BF16, name="xt_bf")
                        nc.vector.tensor_copy(out=xt_bf[:], in_=xt_ps[:].rearrange("p (a b) -> p a b", a=2))
                        qk_ps = ps_qk.tile([K, 512], F32, space="PSUM")
                        for f in range(4):
                            for ci in range(2):
                                nc.tensor.matmul(
                                    out=qk_ps[:, f * K:(f + 1) * K],
                                    lhsT=wqkT_sb[:, ci, f * K:(f + 1) * K],
                                    rhs=xt_bf[:, ci],
                                    start=(ci == 0), stop=(ci == 1),
                                )
                        qkT = pa_qk.tile([K, 512], BF16)
                        nc.vector.tensor_copy(out=qkT[:], in_=qk_ps[:])
                        v_ps = ps_v.tile([K, C], F32, space="PSUM")
                        for ci in range(2):
                            nc.tensor.matmul(
                                out=v_ps[:], lhsT=xt_bf[:, ci], rhs=wvT_sb[:, ci],
                                start=(ci == 0), stop=(ci == 1),
                            )
                        v_bf = pa_v.tile([K, C], BF16)
                        if bz:
                            nc.vector.tensor_copy(out=v_bf[:], in_=v_ps[:])
                        else:
                            nc.vector.tensor_tensor(out=v_bf[:], in0=v_ps[:], in1=bv_sb[:], op=mybir.AluOpType.add)
                        # scores + exp per group; row-sums via PE ones-matmuls
                        expT = pa_p.tile([K, 8, K], BF16, name="expT")
                        l_ps = ps_r.tile([K, 8], F32, space="PSUM", name="l_ps", tag="rps")
                        for g in range(4):
                            sT_ps = ps_sT.tile([K, 2, 512], F32, space="PSUM")
                            for h2 in range(2):
                                h = g * 2 + h2
                                ft_q, pr_q = h // 4, (h % 4) * DH
                                ft_k = 2 + h // 4
                                nc.tensor.matmul(
                                    out=sT_ps[:, h2, 0:K],
                                    lhsT=qkT[pr_q:pr_q + DH, ft_k * K:(ft_k + 1) * K],
                                    rhs=qkT[pr_q:pr_q + DH, ft_q * K:(ft_q + 1) * K],
                                    start=True, stop=True,
                                    tile_position=(pr_q, 0),
                                )
                            nc.scalar.activation(
                                out=expT[:, g * 2:g * 2 + 2, :], in_=sT_ps[:, :, 0:K],
                                func=mybir.ActivationFunctionType.Exp,
                            )
                        for h in range(8):
                            nc.tensor.matmul(
                                out=l_ps[:, h:h + 1], lhsT=expT[:, h, :], rhs=ones_col[:],
                                start=True, stop=True,
                            )
                        # r_col -> transpose -> E-expand to [K, 2, K]
                        r_col = pa_r.tile([K, K], BF16, name="r_col")
                        nc.vector.memset(r_col[:], 0.0)
                        with nc.allow_low_precision(reason="softmax recip in bf16"):
                            nc.vector.reciprocal(out=r_col[:, 0:8], in_=l_ps[:])
                        rT_ps = ps_r.tile([K, K], BF16, space="PSUM", name="rT_ps", tag="rps")
                        nc.tensor.transpose(out=rT_ps[:], in_=r_col[:], identity=ident[:])
                        rT_sb = pa_r.tile([K, K], BF16, name="rT_sb")
                        nc.vector.tensor_copy(out=rT_sb[:], in_=rT_ps[:])
                        re_ps = ps_r.tile([K, 2, K], F32, space="PSUM", name="re_ps", tag="rps")
                        for cch in range(2):
                            nc.tensor.matmul(
                                out=re_ps[:, cch, :], lhsT=e128_sb[:, cch, :], rhs=rT_sb[:],
                                start=True, stop=True,
                            )
                        re_sb = pa_r.tile([K, 2, K], BF16, name="re_sb")
                        nc.vector.tensor_copy(out=re_sb[:], in_=re_ps[:])
                        # av (unnormalized) then normalize in the PSUM->SBUF mult
                        ao_ps = ps_ao.tile([K, 2, K], F32, space="PSUM")
                        for h in range(8):
                            chunk, pr_o = h // 4, (h % 4) * DH
                            nc.tensor.matmul(
                                out=ao_ps[pr_o:pr_o + DH, chunk, :],
                                lhsT=v_bf[:, h * DH:(h + 1) * DH],
                                rhs=expT[:, h, :],
                                start=True, stop=True,
                                tile_position=(0, pr_o),
                            )
                        ao_bf = pa_ao.tile([K, 2, K], BF16)
                        nc.vector.tensor_tensor(out=ao_bf[:], in0=ao_ps[:], in1=re_sb[:], op=mybir.AluOpType.mult)
                        d_ps = ps_d.tile([K, C], F32, space="PSUM")
                        for ci in range(2):
                            nc.tensor.matmul(
                                out=d_ps[:], lhsT=ao_bf[:, ci], rhs=wpT_sb[:, ci],
                                start=(ci == 0), stop=(ci == 1),
                            )
                        delta = pa_d.tile([K, C], F32)
                        if bz:
                            nc.vector.tensor_copy(out=delta[:], in_=d_ps[:])
                        else:
                            nc.vector.tensor_tensor(out=delta[:], in0=d_ps[:], in1=pb_sb[:], op=mybir.AluOpType.add)
                        sp = pa_sm.tile([K, 1], I32, name="sp")
                        nc.sync.dma_start(out=sp[:, 0:1], in_=spos2[p, :].rearrange("(p one) -> p one", one=1))
                        nc.gpsimd.indirect_dma_start(
                            out=send[:], out_offset=bass.IndirectOffsetOnAxis(ap=sp[:, :1], axis=0),
                            in_=delta[:], in_offset=None,
                        )

                for g0 in range(0, n_a, sg):
                    g1 = min(g0 + sg, n_a)
                    ns = g1 - g0
                    mvs = pa_sm.tile([K, sg, 2], F32, name="mvs")
                    st = {"mvs": mvs, "feat": {}}
                    for p in range(g0, g1):
                        _a1_gather(p, p - g0, st)
                    sds = pa_sm.tile([K, sg], F32, name="sds")
                    nc.scalar.activation(out=sds[:, 0:ns], in_=mvs[:, 0:ns, 1:2], func=mybir.ActivationFunctionType.Sqrt, bias=eps_t[:, :1])
                    rstds = pa_sm.tile([K, sg], F32, name="rstds")
                    nc.vector.reciprocal(out=rstds[:, 0:ns], in_=sds[:, 0:ns])
                    st["rstds"] = rstds
                    xn_tiles = {}
                    for p in range(g0, g1):
                        _a1_norm(p, p - g0, st, xn_tiles)
                    for p in range(g0, g1):
                        _a2_body(p, xn_tiles)

            # ================= exchange =================
            if use_a2a:
                nc.gpsimd.collective_compute(
                    "AllToAll", mybir.AluOpType.bypass,
                    replica_groups=[list(range(NCORE))],
                    ins=[send[:]], outs=[recv[:]],
                )
            else:
                nc.sync.dma_start(out=recv[:], in_=send[:])

            # ================= Phase B: original-order slab =================
            with (
                tc.tile_pool(name="pb_sm", bufs=6) as pb_sm,
                tc.tile_pool(name="pb_feat", bufs=3) as pb_feat,
                tc.tile_pool(name="pb_x2", bufs=2 * sg) as pb_x2,
                tc.tile_pool(name="pb_xn", bufs=2 * sg) as pb_xn,
                tc.tile_pool(name="pb_g", bufs=2) as pb_g,
                tc.tile_pool(name="pb_o", bufs=3) as pb_o,
                tc.tile_pool(name="ps_xt2", bufs=2, space="PSUM") as ps_xt2,
                tc.tile_pool(name="ps_h", bufs=3, space="PSUM") as ps_h,
                tc.tile_pool(name="ps_y", bufs=2, space="PSUM") as ps_y,
            ):
                n_b = ppc_b if do_b else 0

                def _b1_gather(t, i, st, x2_tiles):
                        gi = pb_sm.tile([K, 1], I32)
                        nc.sync.dma_start(out=gi[:, 0:1], in_=gidx2[t, :].rearrange("(p one) -> p one", one=1))
                        dl = pb_feat.tile([K, C], F32, name="dl")
                        nc.gpsimd.indirect_dma_start(
                            out=dl[:], out_offset=None, in_=recv[:],
                            in_offset=bass.IndirectOffsetOnAxis(ap=gi[:, :1], axis=0),
                        )
                        ft = pb_feat.tile([K, C], F32, name="ft")
                        nc.sync.dma_start(out=ft[:], in_=feat_slab[t * K:(t + 1) * K, :])
                        x2 = pb_x2.tile([K, C], F32, name="x2")
                        nc.vector.tensor_tensor(out=x2[:], in0=ft[:], in1=dl[:], op=mybir.AluOpType.add)
                        x2_tiles[t] = x2
                        stats2 = pb_sm.tile([K, 6], F32)
                        nc.vector.bn_stats(out=stats2[:], in_=x2[:])
                        nc.vector.bn_aggr(out=st["mvs"][:, i, :], in_=stats2[:])

                def _b1_norm(t, i, st, x2_tiles, xn2_tiles):
                        xn2 = pb_xn.tile([K, C], BF16, name="xn2")
                        nc.vector.tensor_scalar(
                            out=xn2[:], in0=x2_tiles[t][:], scalar1=st["mvs"][:, i, 0:1], scalar2=st["rstds"][:, i:i + 1],
                            op0=mybir.AluOpType.subtract, op1=mybir.AluOpType.mult,
                        )
                        xn2_tiles[t] = xn2

                def _b2_body(t, x2_tiles, xn2_tiles):
                        xn2 = xn2_tiles[t]
                        xt2_ps = ps_xt2.tile([K, C], BF16, space="PSUM")
                        nc.tensor.transpose(out=xt2_ps[:, 0:K], in_=xn2[:, 0:K], identity=ident[:])
                        nc.tensor.transpose(out=xt2_ps[:, K:C], in_=xn2[:, K:C], identity=ident[:])
                        xt2_bf = pb_g.tile([K, 2, K], BF16, name="xt2_bf")
                        nc.vector.tensor_copy(out=xt2_bf[:], in_=xt2_ps[:].rearrange("p (a b) -> p a b", a=2))
                        y_ps = ps_y.tile([K, C], F32, space="PSUM")
                        g_bf = pb_g.tile([K, 8, K], BF16, name="g_bf")
                        for kk in range(2):
                            h_ps = ps_h.tile([K, 4, K], F32, space="PSUM")
                            for sub in range(4):
                                k = kk * 4 + sub
                                for ci in range(2):
                                    nc.tensor.matmul(
                                        out=h_ps[:, sub, :], lhsT=w1T_sb[:, ci, k * K:(k + 1) * K],
                                        rhs=xt2_bf[:, ci], start=(ci == 0), stop=(ci == 1),
                                    )
                            if bz:
                                nc.scalar.activation(
                                    out=g_bf[:, kk * 4:kk * 4 + 4, :], in_=h_ps[:],
                                    func=GELU_FUNC,
                                )
                            else:
                                for sub in range(4):
                                    k = kk * 4 + sub
                                    nc.scalar.activation(
                                        out=g_bf[:, k, :], in_=h_ps[:, sub, :],
                                        func=GELU_FUNC, bias=b1_sb[:, k:k + 1],
                                    )
                        for k in range(8):
                            nc.tensor.matmul(
                                out=y_ps[:], lhsT=g_bf[:, k, :], rhs=w2T_sb[:, k],
                                start=(k == 0), stop=(k == 7),
                            )
                        out_sb = pb_o.tile([K, C], F32, name="out_sb")
                        nc.vector.tensor_tensor(out=out_sb[:], in0=y_ps[:], in1=x2_tiles[t][:], op=mybir.AluOpType.add)
                        if not bz:
                            nc.vector.tensor_tensor(out=out_sb[:], in0=out_sb[:], in1=b2_sb[:], op=mybir.AluOpType.add)
                        nc.sync.dma_start(out=out[t * K:(t + 1) * K, :], in_=out_sb[:])

                for g0 in range(0, n_b, sg):
                    g1 = min(g0 + sg, n_b)
                    ns = g1 - g0
                    mvs2 = pb_sm.tile([K, sg, 2], F32, name="mvs2")
                    st = {"mvs": mvs2}
                    x2_tiles = {}
                    xn2_tiles = {}
                    for t in range(g0, g1):
                        _b1_gather(t, t - g0, st, x2_tiles)
                    sds2 = pb_sm.tile([K, sg], F32, name="sds2")
                    nc.scalar.activation(out=sds2[:, 0:ns], in_=mvs2[:, 0:ns, 1:2], func=mybir.ActivationFunctionType.Sqrt, bias=eps_t[:, :1])
                    rstds2 = pb_sm.tile([K, sg], F32, name="rstds2")
                    nc.vector.reciprocal(out=rstds2[:, 0:ns], in_=sds2[:, 0:ns])
                    st["rstds"] = rstds2
                    for t in range(g0, g1):
                        _b1_norm(t, t - g0, st, x2_tiles, xn2_tiles)
                    for t in range(g0, g1):
                        _b2_body(t, x2_tiles, xn2_tiles)

    return nc


def kernel(**inputs):
    feat = np.ascontiguousarray(np.asarray(inputs["feat"], dtype=np.float32))
    order = np.asarray(inputs["order"]).astype(np.int64)
    qkv_w = np.asarray(inputs["qkv_w"], dtype=np.float32)
    qkv_b = np.asarray(inputs["qkv_b"], dtype=np.float32)
    proj_w = np.asarray(inputs["proj_w"], dtype=np.float32)
    proj_b = np.asarray(inputs["proj_b"], dtype=np.float32)
    ln1_g = np.asarray(inputs["ln1_g"], dtype=np.float32)
    ln1_b = np.asarray(inputs["ln1_b"], dtype=np.float32)
    ln2_g = np.asarray(inputs["ln2_g"], dtype=np.float32)
    ln2_b = np.asarray(inputs["ln2_b"], dtype=np.float32)
    mlp_w1 = np.asarray(inputs["mlp_w1"], dtype=np.float32)
    mlp_b1 = np.asarray(inputs["mlp_b1"], dtype=np.float32)
    mlp_w2 = np.asarray(inputs["mlp_w2"], dtype=np.float32)
    mlp_b2 = np.asarray(inputs["mlp_b2"], dtype=np.float32)

    # ---- host routing from `order` (index-only; all data stays on device) ----
    dest = (order // S).astype(np.int64)
    counts = np.zeros((NCORE, NCORE), np.int64)
    for c in range(NCORE):
        counts[c] = np.bincount(dest[c * S:(c + 1) * S], minlength=NCORE)
    bucket = int(-(-counts.max() // 16) * 16)

    send_pos = np.empty(N, np.int32)
    g_idx = np.empty(N, np.int32)
    for c in range(NCORE):
        d_c = dest[c * S:(c + 1) * S]
        for j in range(NCORE):
            idxs = np.nonzero(d_c == j)[0]
            k = np.arange(len(idxs), dtype=np.int32)
            send_pos[c * S + idxs] = j * bucket + k
            g_idx[order[c * S + idxs]] = c * bucket + k

    # ---- weight prep: fold LN affine + attention scale into matmul weights ----
    wqkv = qkv_w * ln1_g[None, :]
    bqkv = qkv_b + qkv_w @ ln1_b
    wqkv[0:C] *= SCALE
    bqkv[0:C] *= SCALE
    wqkT = np.ascontiguousarray(wqkv[0:2 * C].T)          # [256, 512]
    wvT = np.ascontiguousarray(wqkv[2 * C:3 * C].T)       # [256, 256]
    bqk = np.ascontiguousarray(bqkv[0:2 * C].reshape(4, K).T)   # [128, 4]
    bv = bqkv[2 * C:3 * C]
    wpT = np.ascontiguousarray(proj_w.T)                  # [256, 256]
    w1 = mlp_w1 * ln2_g[None, :]
    b1v = mlp_b1 + mlp_w1 @ ln2_b
    w1T = np.ascontiguousarray(w1.T)                      # [256, 1024]
    b1 = np.ascontiguousarray(b1v.reshape(8, K).T)        # [128, 8]
    w2T = np.ascontiguousarray(mlp_w2.T)                  # [1024, 256]

    bz = not (bqkv.any() or proj_b.any() or b1v.any() or mlp_b2.any())

    key = (bucket, bz)
    if key not in _prog_cache:
        _prog_cache[key] = _build_program(bucket, bz=bz)
    nc = _prog_cache[key]

    # head-expansion matrix: r_exp[p, c, t] = sum_r E128[c][r, p] * rT[r, t]
    # with rT row r = 1/l for head r (r < 8); E128[c][r, p] = (r == 4c + p//32)
    e128 = np.zeros((K, 2, K), np.float32)
    for cch in range(2):
        for p_ in range(K):
            r = 4 * cch + p_ // DH
            e128[r, cch, p_] = 1.0

    import ml_dtypes
    to_bf16 = lambda a: np.ascontiguousarray(a).astype(ml_dtypes.bfloat16)

    common = {
        "feat": feat, "e128": to_bf16(e128),
        "wqkT": to_bf16(wqkT), "wvT": to_bf16(wvT), "bqk": bqk, "bv": bv,
        "wpT": to_bf16(wpT), "pb": proj_b,
        "w1T": to_bf16(w1T), "b1": b1, "w2T": to_bf16(w2T), "b2": mlp_b2,
    }
    in_maps = []
    for c in range(NCORE):
        sl = slice(c * S, (c + 1) * S)
        in_maps.append({
            **common,
            "feat_slab": feat[sl],
            "ordi": order[sl].astype(np.int32),
            "spos": send_pos[sl],
            "gidx": g_idx[sl],
        })

    res = run_bass_kernel_spmd(nc, in_maps, core_ids=list(range(NCORE)))
    return np.concatenate([res.results[c]["out"] for c in range(NCORE)], axis=0)



# revision 1
# speedup vs baseline: 1.3550x; 1.3550x over previous
"""Trainium2 Bass kernel for nn_EncoderLayer_64175401337444 (sparse_attention).

Strategy (8 NeuronCores, data-parallel over patches):
  The whole layer is pointwise-per-token EXCEPT the per-patch attention, and
  the serialized `order` gather / `inverse` scatter are inverse permutations.
  So we run the entire front half (LN1 -> QKV -> patch attention -> proj) in
  SERIALIZED order: core c owns serialized positions [c*16384, (c+1)*16384)
  = 128 patches, gathering its rows from a replicated feat buffer with
  indirect DMA.  The attention branch output ("delta", pre-residual) is then
  exchanged to ORIGINAL row order with a single on-device AllToAll (host
  precomputes the routing from `order`), after which the back half
  (residual -> LN2 -> MLP -> residual) runs on each core's contiguous slab of
  original rows, and each core emits its contiguous slice of the final output.

  LN gains/biases are folded into the adjacent matmul weights on the host;
  matmuls run in bf16 with f32 PSUM accumulation; LN statistics, softmax
  normalization, residuals and the exchanged delta stay f32.
"""
import sys

sys.path.insert(0, "/opt/trn_rl_repo")

import numpy as np

import concourse.bass as bass
import concourse.tile as tile
from concourse import mybir
from concourse.bass_utils import run_bass_kernel_spmd
from concourse.masks import make_identity

# ---------------------------------------------------------------------------
# Workaround for this walrus build accepting at most ONE sync wait per
# instruction: after Tile finishes scheduling, split any multi-wait
# instruction into single-wait same-engine NoOps placed immediately before it.
_uid = [0]


def _split_multi_waits(nc):
    register = getattr(nc, "register_instruction", None)
    for fn in nc.m.functions:
        for bb in fn.blocks:
            insts = bb.instructions
            if not any(
                i.sync_info is not None and len(i.sync_info.on_wait) > 1
                for i in insts
            ):
                continue
            new_list = []
            for inst in insts:
                si = inst.sync_info
                if si is not None and len(si.on_wait) > 1:
                    waits = list(si.on_wait)
                    for w in waits[:-1]:
                        _uid[0] += 1
                        nop = mybir.InstNoOp(
                            name=f"I-waitsplit-{_uid[0]}", ins=[], outs=[]
                        )
                        nop.engine = inst.engine
                        nop.sync_info = mybir.SyncInfo(on_wait=[w], on_update=[])
                        if register is not None:
                            register(nop, overwrite=True)
                        new_list.append(nop)
                    inst.sync_info = mybir.SyncInfo(
                        on_wait=[waits[-1]], on_update=list(si.on_update)
                    )
                new_list.append(inst)
            bb.instructions.clear()
            for inst in new_list:
                bb.instructions.append(inst)


if not getattr(tile.TileContext, "_wait_split_patched", False):
    _orig_dab = tile.TileContext._drain_and_barrier

    def _dab_patched(self, tick_clock, wait_clock):
        _orig_dab(self, tick_clock, wait_clock)
        _split_multi_waits(self.nc)

    tile.TileContext._drain_and_barrier = _dab_patched
    tile.TileContext._wait_split_patched = True

# ---------------------------------------------------------------------------

N = 131072
C = 256
H = 8
K = 128          # patch size == SBUF partition count
HID = 1024
NCORE = 8
S = N // NCORE   # 16384 rows per core
PPC = S // K     # 128 patches per core
DH = C // H      # 32
SCALE = DH ** -0.5
LN_EPS = 1e-5
GELU_FUNC = mybir.ActivationFunctionType.Gelu

F32 = mybir.dt.float32
BF16 = mybir.dt.bfloat16
I32 = mybir.dt.int32

_prog_cache = {}


def _build_program(bucket, ppc_a=PPC, ppc_b=PPC, use_a2a=True, do_a=True, do_b=True, bz=True, sg=16):
    tot = NCORE * bucket
    nc = bass.Bass()

    feat = nc.declare_dram_parameter("feat", [N, C], F32, isOutput=False)
    feat_slab = nc.declare_dram_parameter("feat_slab", [S, C], F32, isOutput=False)
    ordi = nc.declare_dram_parameter("ordi", [S], I32, isOutput=False)
    spos = nc.declare_dram_parameter("spos", [S], I32, isOutput=False)
    gidx = nc.declare_dram_parameter("gidx", [S], I32, isOutput=False)
    wqkT = nc.declare_dram_parameter("wqkT", [C, 512], BF16, isOutput=False)
    wvT = nc.declare_dram_parameter("wvT", [C, C], BF16, isOutput=False)
    bqk = nc.declare_dram_parameter("bqk", [K, 4], F32, isOutput=False)
    bv = nc.declare_dram_parameter("bv", [C], F32, isOutput=False)
    wpT = nc.declare_dram_parameter("wpT", [C, C], BF16, isOutput=False)
    pb = nc.declare_dram_parameter("pb", [C], F32, isOutput=False)
    w1T = nc.declare_dram_parameter("w1T", [C, HID], BF16, isOutput=False)
    b1 = nc.declare_dram_parameter("b1", [K, 8], F32, isOutput=False)
    w2T = nc.declare_dram_parameter("w2T", [HID, C], BF16, isOutput=False)
    b2 = nc.declare_dram_parameter("b2", [C], F32, isOutput=False)
    e128 = nc.declare_dram_parameter("e128", [K, 2, K], BF16, isOutput=False)
    out = nc.declare_dram_parameter("out", [S, C], F32, isOutput=True)

    send = nc.dram_tensor("send", [tot, C], F32)
    recv = nc.dram_tensor("recv", [tot, C], F32)

    ordi2 = ordi.rearrange("(p k) -> p k", k=K)
    spos2 = spos.rearrange("(p k) -> p k", k=K)
    gidx2 = gidx.rearrange("(p k) -> p k", k=K)

    with tile.TileContext(nc) as tc:
        with tc.tile_pool(name="consts", bufs=1) as consts:
            wqkT_sb = consts.tile([K, 2, 512], BF16)
            nc.sync.dma_start(out=wqkT_sb[:], in_=wqkT.rearrange("(k p) f -> p k f", p=K))
            wvT_sb = consts.tile([K, 2, C], BF16)
            nc.sync.dma_start(out=wvT_sb[:], in_=wvT.rearrange("(k p) f -> p k f", p=K))
            wpT_sb = consts.tile([K, 2, C], BF16)
            nc.sync.dma_start(out=wpT_sb[:], in_=wpT.rearrange("(k p) f -> p k f", p=K))
            w1T_sb = consts.tile([K, 2, HID], BF16)
            nc.sync.dma_start(out=w1T_sb[:], in_=w1T.rearrange("(k p) f -> p k f", p=K))
            w2T_sb = consts.tile([K, 8, C], BF16)
            nc.sync.dma_start(out=w2T_sb[:], in_=w2T.rearrange("(k p) f -> p k f", p=K))
            bqk_sb = consts.tile([K, 4], F32)
            nc.sync.dma_start(out=bqk_sb[:], in_=bqk[:])
            b1_sb = consts.tile([K, 8], F32)
            nc.sync.dma_start(out=b1_sb[:], in_=b1[:])
            e128_sb = consts.tile([K, 2, K], BF16)
            nc.sync.dma_start(out=e128_sb[:], in_=e128[:])

            def _bcast(handle):
                a = handle[:]
                return bass.AP(tensor=a.tensor, offset=a.offset, ap=[[0, K]] + list(a.ap))

            bv_sb = consts.tile([K, C], F32)
            nc.sync.dma_start(out=bv_sb[:], in_=_bcast(bv))
            pb_sb = consts.tile([K, C], F32)
            nc.sync.dma_start(out=pb_sb[:], in_=_bcast(pb))
            b2_sb = consts.tile([K, C], F32)
            nc.sync.dma_start(out=b2_sb[:], in_=_bcast(b2))
            ident = consts.tile([K, K], BF16)
            make_identity(nc, ident[:])
            eps_t = consts.tile([K, 1], F32)
            nc.vector.memset(eps_t[:], LN_EPS)
            ones_col = consts.tile([K, 1], BF16)
            nc.vector.memset(ones_col[:], 1.0)

            # ================= Phase A: serialized patches =================
            with (
                tc.tile_pool(name="pa_sm", bufs=6) as pa_sm,
                tc.tile_pool(name="pa_feat", bufs=2 * sg) as pa_feat,
                tc.tile_pool(name="pa_xn", bufs=2 * sg) as pa_xn,
                tc.tile_pool(name="pa_qk", bufs=2) as pa_qk,
                tc.tile_pool(name="pa_v", bufs=2) as pa_v,
                tc.tile_pool(name="pa_p", bufs=2) as pa_p,
                tc.tile_pool(name="pa_r", bufs=2) as pa_r,
                tc.tile_pool(name="pa_ao", bufs=2) as pa_ao,
                tc.tile_pool(name="pa_d", bufs=2) as pa_d,
                tc.tile_pool(name="ps_xt", bufs=1, space="PSUM") as ps_xt,
                tc.tile_pool(name="ps_qk", bufs=1, space="PSUM") as ps_qk,
                tc.tile_pool(name="ps_v", bufs=1, space="PSUM") as ps_v,
                tc.tile_pool(name="ps_sT", bufs=1, space="PSUM") as ps_sT,
                tc.tile_pool(name="ps_r", bufs=1, space="PSUM") as ps_r,
                tc.tile_pool(name="ps_ao", bufs=1, space="PSUM") as ps_ao,
                tc.tile_pool(name="ps_d", bufs=1, space="PSUM") as ps_d,
            ):
                n_a = ppc_a if do_a else 0

                def _a1_gather(p, i, st):
                        idx = pa_sm.tile([K, 1], I32)
                        nc.sync.dma_start(out=idx[:, 0:1], in_=ordi2[p, :].rearrange("(p one) -> p one", one=1))
                        feat_s = pa_feat.tile([K, C], F32)
                        nc.gpsimd.indirect_dma_start(
                            out=feat_s[:], out_offset=None, in_=feat[:],
                            in_offset=bass.IndirectOffsetOnAxis(ap=idx[:, :1], axis=0),
                        )
                        stats = pa_sm.tile([K, 6], F32)
                        nc.vector.bn_stats(out=stats[:], in_=feat_s[:])
                        nc.vector.bn_aggr(out=st["mvs"][:, i, :], in_=stats[:])
                        st["feat"][p] = feat_s

                def _a1_norm(p, i, st, xn_tiles):
                        xn = pa_xn.tile([K, C], BF16, name="xn")
                        nc.vector.tensor_scalar(
                            out=xn[:], in0=st["feat"][p][:], scalar1=st["mvs"][:, i, 0:1], scalar2=st["rstds"][:, i:i + 1],
                            op0=mybir.AluOpType.subtract, op1=mybir.AluOpType.mult,
                        )
                        xn_tiles[p] = xn

                def _a2_body(p, xn_tiles):
                        xn = xn_tiles[p]
                        xt_ps = ps_xt.tile([K, C], BF16, space="PSUM")
                        nc.tensor.transpose(out=xt_ps[:, 0:K], in_=xn[:, 0:K], identity=ident[:])
                        nc.tensor.transpose(out=xt_ps[:, K:C], in_=xn[:, K:C], identity=ident[:])
                        xt_bf = pa_p.tile([K, 2, K], BF16, name="xt_bf")
                        nc.vector.tensor_copy(out=xt_bf[:], in_=xt_ps[:].rearrange("p (a b) -> p a b", a=2))
                        qk_ps = ps_qk.tile([K, 512], F32, space="PSUM")
                        for f in range(4):
                            for ci in range(2):
                                nc.tensor.matmul(
                                    out=qk_ps[:, f * K:(f + 1) * K],
                                    lhsT=wqkT_sb[:, ci, f * K:(f + 1) * K],
                                    rhs=xt_bf[:, ci],
                                    start=(ci == 0), stop=(ci == 1),
                                )
                        qkT = pa_qk.tile([K, 512], BF16)
                        nc.vector.tensor_copy(out=qkT[:], in_=qk_ps[:])
                        v_ps = ps_v.tile([K, C], F32, space="PSUM")
                        for ci in range(2):
                            nc.tensor.matmul(
                                out=v_ps[:], lhsT=xt_bf[:, ci], rhs=wvT_sb[:, ci],
                                start=(ci == 0), stop=(ci == 1),
                            )
                        v_bf = pa_v.tile([K, C], BF16)
                        if bz:
                            nc.vector.tensor_copy(out=v_bf[:], in_=v_ps[:])
                        else:
                            nc.vector.tensor_tensor(out=v_bf[:], in0=v_ps[:], in1=bv_sb[:], op=mybir.AluOpType.add)
                        # scores + exp per group; row-sums via PE ones-matmuls
                        expT = pa_p.tile([K, 8, K], BF16, name="expT")
                        l_ps = ps_r.tile([K, 8], F32, space="PSUM", name="l_ps", tag="rps")
                        for g in range(4):
                            sT_ps = ps_sT.tile([K, 2, 512], F32, space="PSUM")
                            for h2 in range(2):
                                h = g * 2 + h2
                                ft_q, pr_q = h // 4, (h % 4) * DH
                                ft_k = 2 + h // 4
                                nc.tensor.matmul(
                                    out=sT_ps[:, h2, 0:K],
                                    lhsT=qkT[pr_q:pr_q + DH, ft_k * K:(ft_k + 1) * K],
                                    rhs=qkT[pr_q:pr_q + DH, ft_q * K:(ft_q + 1) * K],
                                    start=True, stop=True,
                                    tile_position=(pr_q, 0),
                                )
                            nc.scalar.activation(
                                out=expT[:, g * 2:g * 2 + 2, :], in_=sT_ps[:, :, 0:K],
                                func=mybir.ActivationFunctionType.Exp,
                            )
                        for h in range(8):
                            nc.tensor.matmul(
                                out=l_ps[:, h:h + 1], lhsT=expT[:, h, :], rhs=ones_col[:],
                                start=True, stop=True,
                            )
                        # r_col -> transpose -> E-expand to [K, 2, K]
                        r_col = pa_r.tile([K, K], BF16, name="r_col")
                        nc.vector.memset(r_col[:], 0.0)
                        with nc.allow_low_precision(reason="softmax recip in bf16"):
                            nc.vector.reciprocal(out=r_col[:, 0:8], in_=l_ps[:])
                        rT_ps = ps_r.tile([K, K], BF16, space="PSUM", name="rT_ps", tag="rps")
                        nc.tensor.transpose(out=rT_ps[:], in_=r_col[:], identity=ident[:])
                        rT_sb = pa_r.tile([K, K], BF16, name="rT_sb")
                        nc.vector.tensor_copy(out=rT_sb[:], in_=rT_ps[:])
                        re_ps = ps_r.tile([K, 2, K], F32, space="PSUM", name="re_ps", tag="rps")
                        for cch in range(2):
                            nc.tensor.matmul(
                                out=re_ps[:, cch, :], lhsT=e128_sb[:, cch, :], rhs=rT_sb[:],
                                start=True, stop=True,
                            )
                        re_sb = pa_r.tile([K, 2, K], BF16, name="re_sb")
                        nc.vector.tensor_copy(out=re_sb[:], in_=re_ps[:])
                        # av (unnormalized) then normalize in the PSUM->SBUF mult
                        ao_ps = ps_ao.tile([K, 2, K], F32, space="PSUM")
                        for h in range(8):
                            chunk, pr_o = h // 4, (h % 4) * DH
                            nc.tensor.matmul(
                                out=ao_ps[pr_o:pr_o + DH, chunk, :],
                                lhsT=v_bf[:, h * DH:(h + 1) * DH],
                                rhs=expT[:, h, :],
                                start=True, stop=True,
                                tile_position=(0, pr_o),
                            )
                        ao_bf = pa_ao.tile([K, 2, K], BF16)
                        nc.vector.tensor_tensor(out=ao_bf[:], in0=ao_ps[:], in1=re_sb[:], op=mybir.AluOpType.mult)
                        d_ps = ps_d.tile([K, C], F32, space="PSUM")
                        for ci in range(2):
                            nc.tensor.matmul(
                                out=d_ps[:], lhsT=ao_bf[:, ci], rhs=wpT_sb[:, ci],
                                start=(ci == 0), stop=(ci == 1),
                            )
                        delta = pa_d.tile([K, C], F32)
                        if bz:
                            nc.vector.tensor_copy(out=delta[:], in_=d_ps[:])
                        else:
                            nc.vector.tensor_tensor(out=delta[:], in0=d_ps[:], in1=pb_sb[:], op=mybir.AluOpType.add)
                        sp = pa_sm.tile([K, 1], I32, name="sp")
                        nc.sync.dma_start(out=sp[:, 0:1], in_=spos2[p, :].rearrange("(p one) -> p one", one=1))
                        nc.gpsimd.indirect_dma_start(
                            out=send[:], out_offset=bass.IndirectOffsetOnAxis(ap=sp[:, :1], axis=0),
                            in_=delta[:], in_offset=None,
                        )

                for g0 in range(0, n_a, sg):
                    g1 = min(g0 + sg, n_a)
                    ns = g1 - g0
                    mvs = pa_sm.tile([K, sg, 2], F32, name="mvs")
                    st = {"mvs": mvs, "feat": {}}
                    for p in range(g0, g1):
                        _a1_gather(p, p - g0, st)
                    sds = pa_sm.tile([K, sg], F32, name="sds")
                    nc.scalar.activation(out=sds[:, 0:ns], in_=mvs[:, 0:ns, 1:2], func=mybir.ActivationFunctionType.Sqrt, bias=eps_t[:, :1])
                    rstds = pa_sm.tile([K, sg], F32, name="rstds")
                    nc.vector.reciprocal(out=rstds[:, 0:ns], in_=sds[:, 0:ns])
                    st["rstds"] = rstds
                    xn_tiles = {}
                    for p in range(g0, g1):
                        _a1_norm(p, p - g0, st, xn_tiles)
                    for p in range(g0, g1):
                        _a2_body(p, xn_tiles)

            # ================= exchange =================
            if use_a2a:
                nc.gpsimd.collective_compute(
                    "AllToAll", mybir.AluOpType.bypass,
                    replica_groups=[list(range(NCORE))],
                    ins=[send[:]], outs=[recv[:]],
                )
            else:
                nc.sync.dma_start(out=recv[:], in_=send[:])

            # ================= Phase B: original-order slab =================
            with (
                tc.tile_pool(name="pb_sm", bufs=6) as pb_sm,
                tc.tile_pool(name="pb_feat", bufs=3) as pb_feat,
                tc.tile_pool(name="pb_x2", bufs=2 * sg) as pb_x2,
                tc.tile_pool(name="pb_xn", bufs=2 * sg) as pb_xn,
                tc.tile_pool(name="pb_g", bufs=2) as pb_g,
                tc.tile_pool(name="pb_o", bufs=3) as pb_o,
                tc.tile_pool(name="ps_xt2", bufs=2, space="PSUM") as ps_xt2,
                tc.tile_pool(name="ps_h", bufs=3, space="PSUM") as ps_h,
                tc.tile_pool(name="ps_y", bufs=2, space="PSUM") as ps_y,
            ):
                n_b = ppc_b if do_b else 0

                def _b1_gather(t, i, st, x2_tiles):
                        gi = pb_sm.tile([K, 1], I32)
                        nc.sync.dma_start(out=gi[:, 0:1], in_=gidx2[t, :].rearrange("(p one) -> p one", one=1))
                        dl = pb_feat.tile([K, C], F32, name="dl")
                        nc.gpsimd.indirect_dma_start(
                            out=dl[:], out_offset=None, in_=recv[:],
                            in_offset=bass.IndirectOffsetOnAxis(ap=gi[:, :1], axis=0),
                        )
                        ft = pb_feat.tile([K, C], F32, name="ft")
                        nc.sync.dma_start(out=ft[:], in_=feat_slab[t * K:(t + 1) * K, :])
                        x2 = pb_x2.tile([K, C], F32, name="x2")
                        nc.vector.tensor_tensor(out=x2[:], in0=ft[:], in1=dl[:], op=mybir.AluOpType.add)
                        x2_tiles[t] = x2
                        stats2 = pb_sm.tile([K, 6], F32)
                        nc.vector.bn_stats(out=stats2[:], in_=x2[:])
                        nc.vector.bn_aggr(out=st["mvs"][:, i, :], in_=stats2[:])

                def _b1_norm(t, i, st, x2_tiles, xn2_tiles):
                        xn2 = pb_xn.tile([K, C], BF16, name="xn2")
                        nc.vector.tensor_scalar(
                            out=xn2[:], in0=x2_tiles[t][:], scalar1=st["mvs"][:, i, 0:1], scalar2=st["rstds"][:, i:i + 1],
                            op0=mybir.AluOpType.subtract, op1=mybir.AluOpType.mult,
                        )
                        xn2_tiles[t] = xn2

                def _b2_body(t, x2_tiles, xn2_tiles):
                        xn2 = xn2_tiles[t]
                        xt2_ps = ps_xt2.tile([K, C], BF16, space="PSUM")
                        nc.tensor.transpose(out=xt2_ps[:, 0:K], in_=xn2[:, 0:K], identity=ident[:])
                        nc.tensor.transpose(out=xt2_ps[:, K:C], in_=xn2[:, K:C], identity=ident[:])
                        xt2_bf = pb_g.tile([K, 2, K], BF16, name="xt2_bf")
                        nc.vector.tensor_copy(out=xt2_bf[:], in_=xt2_ps[:].rearrange("p (a b) -> p a b", a=2))
                        y_ps = ps_y.tile([K, C], F32, space="PSUM")
                        g_bf = pb_g.tile([K, 8, K], BF16, name="g_bf")
                        for kk in range(2):
                            h_ps = ps_h.tile([K, 4, K], F32, space="PSUM")
                            for sub in range(4):
                                k = kk * 4 + sub
                                for ci in range(2):
                                    nc.tensor.matmul(
                                        out=h_ps[:, sub, :], lhsT=w1T_sb[:, ci, k * K:(k + 1) * K],
                                        rhs=xt2_bf[:, ci], start=(ci == 0), stop=(ci == 1),
                                    )
                            if bz:
                                nc.scalar.activation(
                                    out=g_bf[:, kk * 4:kk * 4 + 4, :], in_=h_ps[:],
                                    func=GELU_FUNC,
                                )
                            else:
                                for sub in range(4):
                                    k = kk * 4 + sub
                                    nc.scalar.activation(
                                        out=g_bf[:, k, :], in_=h_ps[:, sub, :],
                                        func=GELU_FUNC, bias=b1_sb[:, k:k + 1],
                                    )
                        for k in range(8):
                            nc.tensor.matmul(
                                out=y_ps[:], lhsT=g_bf[:, k, :], rhs=w2T_sb[:, k],
                                start=(k == 0), stop=(k == 7),
                            )
                        out_sb = pb_o.tile([K, C], F32, name="out_sb")
                        nc.vector.tensor_tensor(out=out_sb[:], in0=y_ps[:], in1=x2_tiles[t][:], op=mybir.AluOpType.add)
                        if not bz:
                            nc.vector.tensor_tensor(out=out_sb[:], in0=out_sb[:], in1=b2_sb[:], op=mybir.AluOpType.add)
                        nc.sync.dma_start(out=out[t * K:(t + 1) * K, :], in_=out_sb[:])

                for g0 in range(0, n_b, sg):
                    g1 = min(g0 + sg, n_b)
                    ns = g1 - g0
                    mvs2 = pb_sm.tile([K, sg, 2], F32, name="mvs2")
                    st = {"mvs": mvs2}
                    x2_tiles = {}
                    xn2_tiles = {}
                    for t in range(g0, g1):
                        _b1_gather(t, t - g0, st, x2_tiles)
                    sds2 = pb_sm.tile([K, sg], F32, name="sds2")
                    nc.scalar.activation(out=sds2[:, 0:ns], in_=mvs2[:, 0:ns, 1:2], func=mybir.ActivationFunctionType.Sqrt, bias=eps_t[:, :1])
                    rstds2 = pb_sm.tile([K, sg], F32, name="rstds2")
                    nc.vector.reciprocal(out=rstds2[:, 0:ns], in_=sds2[:, 0:ns])
                    st["rstds"] = rstds2
                    for t in range(g0, g1):
                        _b1_norm(t, t - g0, st, x2_tiles, xn2_tiles)
                    for t in range(g0, g1):
                        _b2_body(t, x2_tiles, xn2_tiles)

    return nc


def kernel(**inputs):
    feat = np.ascontiguousarray(np.asarray(inputs["feat"], dtype=np.float32))
    order = np.asarray(inputs["order"]).astype(np.int64)
    qkv_w = np.asarray(inputs["qkv_w"], dtype=np.float32)
    qkv_b = np.asarray(inputs["qkv_b"], dtype=np.float32)
    proj_w = np.asarray(inputs["proj_w"], dtype=np.float32)
    proj_b = np.asarray(inputs["proj_b"], dtype=np.float32)
    ln1_g = np.asarray(inputs["ln1_g"], dtype=np.float32)
    ln1_b = np.asarray(inputs["ln1_b"], dtype=np.float32)
    ln2_g = np.asarray(inputs["ln2_g"], dtype=np.float32)
    ln2_b = np.asarray(inputs["ln2_b"], dtype=np.float32)
    mlp_w1 = np.asarray(inputs["mlp_w1"], dtype=np.float32)
    mlp_b1 = np.asarray(inputs["mlp_b1"], dtype=np.float32)
    mlp_w2 = np.asarray(inputs["mlp_w2"], dtype=np.float32)
    mlp_b2 = np.asarray(inputs["mlp_b2"], dtype=np.float32)

    # ---- host routing from `order` (index-only; all data stays on device) ----
    dest = (order // S).astype(np.int64)
    counts = np.zeros((NCORE, NCORE), np.int64)
    for c in range(NCORE):
        counts[c] = np.bincount(dest[c * S:(c + 1) * S], minlength=NCORE)
    bucket = int(-(-counts.max() // 16) * 16)

    send_pos = np.empty(N, np.int32)
    g_idx = np.empty(N, np.int32)
    for c in range(NCORE):
        d_c = dest[c * S:(c + 1) * S]
        for j in range(NCORE):
            idxs = np.nonzero(d_c == j)[0]
            k = np.arange(len(idxs), dtype=np.int32)
            send_pos[c * S + idxs] = j * bucket + k
            g_idx[order[c * S + idxs]] = c * bucket + k

    # ---- weight prep: fold LN affine + attention scale into matmul weights ----
    wqkv = qkv_w * ln1_g[None, :]
    bqkv = qkv_b + qkv_w @ ln1_b
    wqkv[0:C] *= SCALE
    bqkv[0:C] *= SCALE
    wqkT = np.ascontiguousarray(wqkv[0:2 * C].T)          # [256, 512]
    wvT = np.ascontiguousarray(wqkv[2 * C:3 * C].T)       # [256, 256]
    bqk = np.ascontiguousarray(bqkv[0:2 * C].reshape(4, K).T)   # [128, 4]
    bv = bqkv[2 * C:3 * C]
    wpT = np.ascontiguousarray(proj_w.T)                  # [256, 256]
    w1 = mlp_w1 * ln2_g[None, :]
    b1v = mlp_b1 + mlp_w1 @ ln2_b
    w1T = np.ascontiguousarray(w1.T)                      # [256, 1024]
    b1 = np.ascontiguousarray(b1v.reshape(8, K).T)        # [128, 8]
    w2T = np.ascontiguousarray(mlp_w2.T)                  # [1024, 256]

    bz = not (bqkv.any() or proj_b.any() or b1v.any() or mlp_b2.any())

    key = (bucket, bz)
    if key not in _prog_cache:
        _prog_cache[key] = _build_program(bucket, bz=bz)
    nc = _prog_cache[key]

    # head-expansion matrix: r_exp[p, c, t] = sum_r E128[c][r, p] * rT[r, t]
    # with rT row r = 1/l for head r (r < 8); E128[c][r, p] = (r == 4c + p//32)
    e128 = np.zeros((K, 2, K), np.float32)
    for cch in range(2):
        for p_ in range(K):
            r = 4 * cch + p_ // DH
            e128[r, cch, p_] = 1.0

    import ml_dtypes
    to_bf16 = lambda a: np.ascontiguousarray(a).astype(ml_dtypes.bfloat16)

    common = {
        "feat": feat, "e128": to_bf16(e128),
        "wqkT": to_bf16(wqkT), "wvT": to_bf16(wvT), "bqk": bqk, "bv": bv,
        "wpT": to_bf16(wpT), "pb": proj_b,
        "w1T": to_bf16(w1T), "b1": b1, "w2T": to_bf16(w2T), "b2": mlp_b2,
    }
    in_maps = []
    for c in range(NCORE):
        sl = slice(c * S, (c + 1) * S)
        in_maps.append({
            **common,
            "feat_slab": feat[sl],
            "ordi": order[sl].astype(np.int32),
            "spos": send_pos[sl],
            "gidx": g_idx[sl],
        })

    res = run_bass_kernel_spmd(nc, in_maps, core_ids=list(range(NCORE)))
    return np.concatenate([res.results[c]["out"] for c in range(NCORE)], axis=0)

